# revision 106
# baseline (speedup 1.0000x reference)
"""KAN layer (Chebyshev order-7 on tanh(x)) as a Bass/Tile TRN2 kernel.

Math: out[b,o] = sum_{i,k} T_k(tanh(x[b,i])) * W[o,i,k] + bias[o],  k=0..7.

T_0 == 1 folds into an effective bias on the host. The device contracts
the remaining 7*1024 = 7168 (i,k) pairs per output.

Device strategy (data-parallel over batch, 512 rows/core):
- Basis is built on-chip in fp16: u = tanh(x), T_2 = 2u^2 - 1, then the
  even/odd Chebyshev recurrences T_{k+2} = (2 T_2) T_k - T_{k-2} as fp16
  tensor_tensor ops on the DVE (2x perf mode for 2-byte dtypes).
- The matmul runs in fp8e4 (e4m3) with DoubleRow perf mode at half a
  cycle per output row. The two DoubleRow "planes" carry a hi/lo split
  of the weights (Wh = fp8(W*2^12), Wl = fp8(W*2^12 - Wh)) against the
  same fp8 basis tile (stride-0 broadcast rhs), which cancels the
  weight-quantization error. The basis is quantized to fp8 unscaled
  (|T_k| <= ~1 sits fine in e4m3).
- psum accumulates in f32; output = psum * 2^-12 + bias_eff in fp16.
- The PE consumes chunk-major (k2..k7 of contraction-chunk a = itiles
  0-3, then chunk b) so basis production stays ahead; the last two
  levels run otile-outer with the descale+bias+store fused in,
  overlapping the drain. Dummy warm-up matmuls burn the PE p-state
  ramp before the real stream.
- Four "pure" fp8 pairs — (k4,k5) and (k6,k7) at itiles 0-1 — pack two
  k-tiles per DoubleRow instruction (planes = two basis tiles, Wh only,
  no Wl): halves those tiles' PE time and weight bytes for a measured
  rel-err cost of 0.0184 -> 0.0190 against the 0.02 gate.
"""

import sys

sys.path.insert(0, "/opt/trn_rl_repo")

import math

import ml_dtypes
import numpy as np

import concourse.bass as bass  # noqa: F401  (engine types come via bacc)
import concourse.mybir as mybir
from concourse import bacc
from concourse.bass_utils import run_bass_kernel_spmd
from concourse.tile import TileContext

P = 128
N_CORES = 8
BATCH = 4096
B_CORE = BATCH // N_CORES  # 512
IN_F = 1024
OUT_F = 1024
KORD = 7  # Chebyshev T_1..T_7 (T_0 folded into bias)
N_ITILES = IN_F // P  # 8
N_OTILES = OUT_F // P  # 8
CHUNK = 2048  # free-dim chunk: 4 itiles per chunk
N_CHUNKS = 2
SW = 4096.0  # weight fp8 scale (2^12); basis is unscaled (|T_k| <= ~1)
DESCALE = 1.0 / SW

F32 = mybir.dt.float32
F16 = mybir.dt.float16
F8 = mybir.dt.float8e4
DR = mybir.MatmulPerfMode.DoubleRow
ACT_COPY = mybir.ActivationFunctionType.Copy
ACT_TANH = mybir.ActivationFunctionType.Tanh
ACT_SQUARE = mybir.ActivationFunctionType.Square
ACT_ID = mybir.ActivationFunctionType.Identity
MULT = mybir.AluOpType.mult
SUBTRACT = mybir.AluOpType.subtract
ADD = mybir.AluOpType.add

_NC_CACHE = None


def _build():
    """Build + compile the single-core Bass program (SPMD across 8 cores)."""
    global _NC_CACHE
    if _NC_CACHE is not None:
        return _NC_CACHE

    nc = bacc.Bacc("TRN2", target_bir_lowering=False, debug=False)

    # xT16[p, it*512 + b] = x[b, it*128 + p] as fp16, for this core's slice.
    xT16 = nc.declare_dram_parameter("xT16", [P, N_ITILES * B_CORE], F16, isOutput=False)
    # wT[k', it, p, ot*256 + pl*128 + o] = {Wh,Wl}[ot*128+o, it*128+p, k'+1]
    wT = nc.declare_dram_parameter(
        "wT", [KORD, N_ITILES, P, N_OTILES * 2 * P], F8, isOutput=False
    )
    # biasT[p, ot] = bias_eff[ot*128 + p]
    biasT = nc.declare_dram_parameter("biasT", [P, N_OTILES], F32, isOutput=False)
    # Pure-fp8 pair tiles: planes = (Wh_k4, Wh_k5) for itiles 0-1 (no Wl
    # there; measured rel err 0.0184 vs the 0.02 gate). One DoubleRow
    # instruction then contracts both k-tiles at once.
    wPairT = nc.declare_dram_parameter("wPairT", [2, P, N_OTILES * 2 * P], F8, isOutput=False)
    # same for (Wh_k6, Wh_k7) at itiles 0-1
    wPair2T = nc.declare_dram_parameter("wPair2T", [2, P, N_OTILES * 2 * P], F8, isOutput=False)
    # outT[ot, p, b] = out[b, ot*128 + p] fp16
    outT = nc.declare_dram_parameter("outT", [N_OTILES, P, B_CORE], F16, isOutput=True)

    with TileContext(nc) as tc:
        with (
            tc.tile_pool(name="x", bufs=1) as x_pool,
            tc.tile_pool(name="chain", bufs=1) as chain_pool,
            tc.tile_pool(name="tmp", bufs=4) as tmp_pool,
            tc.tile_pool(name="b8", bufs=1) as b8_pool,
            tc.tile_pool(name="w", bufs=20) as w_pool,
            tc.tile_pool(name="osb", bufs=8) as osb_pool,
            tc.tile_pool(name="misc", bufs=1) as misc_pool,
            tc.tile_pool(name="psum", bufs=1, space="PSUM") as psum_pool,
        ):
            # x arrives in 4 pieces of [128, 1024] so tanh/cast/matmul can
            # start early; the first k=1 weight tile is interleaved between
            # x pieces on the (serial) DMA bus.
            PIECE = 1024
            N_PIECES = 4
            x_full = x_pool.tile([P, N_ITILES * B_CORE], F16, name="x_full")
            w_early = {}
            for j in range(N_PIECES):
                sl = slice(j * PIECE, (j + 1) * PIECE)
                nc.sync.dma_start(out=x_full[:, sl], in_=xT16[:, sl])
                if j <= 1:
                    wsb = w_pool.tile([P, N_OTILES * 2 * P], F8, tag="w")
                    nc.sync.dma_start(out=wsb, in_=wT[0, j, :, :])
                    w_early[j] = wsb

            bias_sb = misc_pool.tile([P, N_OTILES], F32, name="bias_sb")
            nc.sync.dma_start(out=bias_sb, in_=biasT[:, :])

            # Warm-up tile: dummy matmuls below burn the PE p-state ramp
            # (0.65/1.2 GHz for the first 3us of busy) before the real
            # stream starts, so k=1 matmuls run at the full 2.4 GHz.
            warm = misc_pool.tile([P, B_CORE], F16, name="warm")
            nc.vector.memset(warm, 0.0)

            ones = misc_pool.tile([P, CHUNK], F16, name="ones")
            nc.vector.memset(ones, 1.0)

            # ---- Chebyshev basis in fp16; fp8 casts on Act ----
            # b8[k][c] holds fp8(T_k) for itiles 4c..4c+3.
            b8 = [[None] * N_CHUNKS for _ in range(KORD + 1)]

            # (k4,k5) and (k6,k7) of chunk a each share one allocation so the
            # pure-pair matmuls can address both as DoubleRow planes
            # (plane stride 2048 within the joint tile).
            b8_45 = b8_pool.tile([P, 2 * CHUNK], F8, name="b8_45")
            b8_67 = b8_pool.tile([P, 2 * CHUNK], F8, name="b8_67")
            pair_dst = {
                (4, 0): b8_45[:, 0:CHUNK],
                (5, 0): b8_45[:, CHUNK : 2 * CHUNK],
                (6, 0): b8_67[:, 0:CHUNK],
                (7, 0): b8_67[:, CHUNK : 2 * CHUNK],
            }

            def cast(k, c, src):
                dst = pair_dst.get((k, c))
                if dst is None:
                    dst = b8_pool.tile([P, CHUNK], F8, name=f"b8_{k}_{c}")
                nc.scalar.activation(dst, src, ACT_COPY)
                b8[k][c] = dst

            # u and b8[1] live as full-width tiles written piecewise so the
            # Act stream can emit tanh_p0, cast1_p0 before tanh_p1 etc.
            u_full = chain_pool.tile([P, N_ITILES * B_CORE], F16, name="u_full")
            b8_1 = b8_pool.tile([P, N_ITILES * B_CORE], F8, name="b8_1")
            b8[1] = [
                b8_1[:, c * CHUNK : (c + 1) * CHUNK] for c in range(N_CHUNKS)
            ]
            # Piece 0: fp8 tanh straight from x so the PE starts ASAP. fp16
            # tanh runs per piece; fp8 copies of pieces 1-3 go to the DVE.
            # Emission order is scheduling priority: sq_a sits between t16
            # pieces so only short ops can delay it, keeping the k=2 chunk-a
            # path (sq_a -> T2a halves -> cast2a halves) as early as possible.
            # Chunk b's square runs on the DVE to keep Act free for casts.
            nc.scalar.activation(b8_1[:, 0:PIECE], x_full[:, 0:PIECE], ACT_TANH)
            for j in (0, 1):
                sl = slice(j * PIECE, (j + 1) * PIECE)
                nc.scalar.activation(u_full[:, sl], x_full[:, sl], ACT_TANH)
            u = [u_full[:, c * CHUNK : (c + 1) * CHUNK] for c in range(N_CHUNKS)]

            sq_a = chain_pool.tile([P, CHUNK], F16, name="sq_a")
            nc.scalar.activation(sq_a, u[0], ACT_SQUARE, scale=math.sqrt(2.0))

            # Pieces 2-3 of the k=1 fp8 basis come straight from Act tanh;
            # their fp16 tanh is only needed by chunk-b's chain (huge slack)
            # and is emitted in the chunk-b section below.
            for j in (2, 3):
                sl = slice(j * PIECE, (j + 1) * PIECE)
                nc.scalar.activation(b8_1[:, sl], x_full[:, sl], ACT_TANH)
            nc.vector.tensor_copy(b8_1[:, PIECE : 2 * PIECE], u_full[:, PIECE : 2 * PIECE])

            # The PE consumes chunk-major (k2a..k7a then k2b..k7b), so the
            # full chunk-a chain is produced first; chunk-b has ~15us slack.
            T2, M = [None] * N_CHUNKS, [None] * N_CHUNKS
            # chunk a: T2 from the Act square, in halves for early casts
            t2a = chain_pool.tile([P, CHUNK], F16, name="T2_a")
            for h in range(2):
                hs = slice(h * PIECE, (h + 1) * PIECE)
                nc.vector.tensor_tensor(
                    out=t2a[:, hs], in0=sq_a[:, hs], in1=ones[:, hs], op=SUBTRACT
                )
            T2[0] = t2a
            b8_2 = b8_pool.tile([P, CHUNK], F8, name="b8_2_0")
            for h in range(2):
                hs = slice(h * PIECE, (h + 1) * PIECE)
                nc.scalar.activation(b8_2[:, hs], t2a[:, hs], ACT_COPY)
            b8[2][0] = b8_2

            Tk = {1: u, 2: T2}
            prev_of = {3: (1, 1), 4: (2, 0), 5: (3, 1), 6: (4, 2), 7: (5, 3)}

            # All chain DVE ops run as [1024] halves: smaller blocking
            # granularity defuses the ack-latency trap where a long op grabs
            # the engine right before a critical dependent becomes ready.
            HALVES = (slice(0, PIECE), slice(PIECE, CHUNK))

            def tt_halved(out_t, in0, in1, op):
                for hs in HALVES:
                    nc.vector.tensor_tensor(
                        out=out_t[:, hs], in0=in0[:, hs], in1=in1[:, hs], op=op
                    )

            def chain_step(k, c):
                # T_{k+2} = (2 T_2) T_k - T_{k-2}: odd from T_1, even from T_2
                src_k, sub_k = prev_of[k]
                tm = tmp_pool.tile([P, CHUNK], F16, tag="tmp")
                tt_halved(tm, M[c], Tk[src_k][c], MULT)
                tk = chain_pool.tile([P, CHUNK], F16, name=f"T{k}_{c}")
                tt_halved(tk, tm, ones if sub_k == 0 else Tk[sub_k][c], SUBTRACT)
                Tk[k].append(tk)
                cast(k, c, tk)

            for k in range(3, KORD + 1):
                Tk[k] = []

            m_a = chain_pool.tile([P, CHUNK], F16, name="M_0")
            tt_halved(m_a, t2a, t2a, ADD)
            M[0] = m_a
            for k in range(3, KORD + 1):
                chain_step(k, 0)

            # chunk b: fp16 tanh (only consumer is this chain), then
            # 2u^2 on the DVE (scalar_tensor_tensor), then -1
            for j in (2, 3):
                sl = slice(j * PIECE, (j + 1) * PIECE)
                nc.scalar.activation(u_full[:, sl], x_full[:, sl], ACT_TANH)
            d_b = chain_pool.tile([P, CHUNK], F16, name="d_b")
            for hs in HALVES:
                nc.vector.scalar_tensor_tensor(
                    out=d_b[:, hs], in0=u[1][:, hs], scalar=2.0, in1=u[1][:, hs],
                    op0=MULT, op1=MULT,
                )
            t2b = chain_pool.tile([P, CHUNK], F16, name="T2_b")
            tt_halved(t2b, d_b, ones, SUBTRACT)
            T2[1] = t2b
            cast(2, 1, t2b)
            m_b = chain_pool.tile([P, CHUNK], F16, name="M_1")
            tt_halved(m_b, t2b, t2b, ADD)
            M[1] = m_b
            for k in range(3, KORD + 1):
                chain_step(k, 1)

            # ---- DoubleRow fp8 matmuls: psum[ot] += (Wh|Wl).T @ (Bh, Bh) ----
            psums = [
                psum_pool.tile([P, B_CORE], F32, name=f"ps_{ot}")
                for ot in range(N_OTILES)
            ]
            def rhs_for(k, c, itl):
                return (
                    b8[k][c][:, itl * B_CORE : (itl + 1) * B_CORE]
                    .unsqueeze(1)
                    .broadcast_to([P, 2, B_CORE])
                )

            def lhs_for(wsb, ot):
                return wsb[:, ot * 2 * P : (ot + 1) * 2 * P].rearrange(
                    "p (two m) -> p two m", two=2
                )

            # p-state warm-up: open-and-close dummy accumulation groups on
            # bank 0; the real k=1 start=True matmul resets it afterwards.
            for _ in range(6):
                nc.tensor.matmul(
                    psums[0],
                    lhsT=warm[:, 0:P],
                    rhs=warm[:, :],
                    start=True,
                    stop=True,
                )

            ITL = N_ITILES // N_CHUNKS
            s = 0
            # k=1 for both chunks first (warm-up while the chain spins up)
            for c in range(N_CHUNKS):
                for itl in range(ITL):
                    it = c * ITL + itl
                    if it in w_early:
                        wsb = w_early[it]
                    else:
                        wsb = w_pool.tile([P, N_OTILES * 2 * P], F8, tag="w")
                        nc.sync.dma_start(out=wsb, in_=wT[0, it, :, :])
                    rhs = rhs_for(1, c, itl)
                    for ot in range(N_OTILES):
                        nc.tensor.matmul(
                            psums[ot],
                            lhsT=lhs_for(wsb, ot),
                            rhs=rhs,
                            start=(s == 0),
                            stop=False,
                            perf_mode=DR,
                        )
                    s += 1

            # chunk-major: all of chunk a's k=2..7, then chunk b's k=2..6
            def mm_block(k, c, skip=()):
                for itl in range(ITL):
                    if itl in skip:
                        continue
                    it = c * ITL + itl
                    wsb = w_pool.tile([P, N_OTILES * 2 * P], F8, tag="w")
                    nc.sync.dma_start(out=wsb, in_=wT[k - 1, it, :, :])
                    rhs = rhs_for(k, c, itl)
                    for ot in range(N_OTILES):
                        nc.tensor.matmul(
                            psums[ot],
                            lhsT=lhs_for(wsb, ot),
                            rhs=rhs,
                            start=False,
                            stop=False,
                            perf_mode=DR,
                        )

            TAIL_LEVELS = (KORD - 1, KORD)  # k=6..7 of chunk b
            w_tail = {}

            def pair_mms(joint_tile, wparam, itl):
                wp = w_pool.tile([P, N_OTILES * 2 * P], F8, tag="w")
                nc.sync.dma_start(out=wp, in_=wparam[itl, :, :])
                rhs = joint_tile[:, :].rearrange("p (two half) -> p two half", two=2)[
                    :, :, itl * B_CORE : (itl + 1) * B_CORE
                ]
                for ot in range(N_OTILES):
                    nc.tensor.matmul(
                        psums[ot],
                        lhsT=lhs_for(wp, ot),
                        rhs=rhs,
                        start=False,
                        stop=False,
                        perf_mode=DR,
                    )

            mm_block(2, 0)
            mm_block(3, 0)
            mm_block(4, 0, skip=(0, 1))
            mm_block(5, 0, skip=(0, 1))
            for itl in (0, 1):
                pair_mms(b8_45, wPairT, itl)
            mm_block(6, 0, skip=(0, 1))
            mm_block(7, 0, skip=(0, 1))
            for itl in (0, 1):
                pair_mms(b8_67, wPair2T, itl)
            for k in range(2, TAIL_LEVELS[0]):
                mm_block(k, 1)
            for kt in TAIL_LEVELS:
                for itl in range(ITL):
                    wsb = w_pool.tile([P, N_OTILES * 2 * P], F8, tag="w")
                    nc.sync.dma_start(out=wsb, in_=wT[kt - 1, ITL + itl, :, :])
                    w_tail[(kt, itl)] = wsb

            # Final levels of chunk b run ot-outer so each psum bank finishes
            # early; the spacing lets every otile's descale+store (and its
            # serial HWDGE descriptor-gen) drain under the remaining matmuls.
            for ot in range(N_OTILES):
                for k in TAIL_LEVELS:
                    for itl in range(ITL):
                        nc.tensor.matmul(
                            psums[ot],
                            lhsT=lhs_for(w_tail[(k, itl)], ot),
                            rhs=rhs_for(k, 1, itl),
                            start=False,
                            stop=(k == KORD and itl == ITL - 1),
                            perf_mode=DR,
                        )
                # ---- descale + bias add + store (fp16) ----
                osb = osb_pool.tile([P, B_CORE], F16, tag="osb")
                nc.scalar.activation(
                    osb,
                    psums[ot],
                    ACT_ID,
                    bias=bias_sb[:, ot : ot + 1],
                    scale=DESCALE,
                )
                nc.sync.dma_start(out=outT[ot, :, :], in_=osb)

    nc.compile()
    _NC_CACHE = nc
    return _NC_CACHE


def _prep_inputs(x, weights, bias_param):
    x = np.asarray(x, dtype=np.float32)
    weights = np.asarray(weights, dtype=np.float32)
    bias_param = np.asarray(bias_param, dtype=np.float32)
    f8 = ml_dtypes.float8_e4m3

    # Weights: [o, i, k] -> hi/lo fp8 at scale 2^12, laid out
    # wT[k', it, p, ot, pl, o] with the last 3 dims contiguous (2KB lines).
    W7 = weights[:, :, 1:] * np.float32(SW)  # [o, i, 7]
    Wh = W7.astype(f8)
    Wl = (W7 - Wh.astype(np.float32)).astype(f8)
    arr = np.stack([Wh, Wl], axis=-1)  # [o_g, i_g, k, pl]
    arr = arr.reshape(N_OTILES, P, N_ITILES, P, KORD, 2)  # [ot, o, it, p, k, pl]
    wT = np.ascontiguousarray(arr.transpose(4, 2, 3, 0, 5, 1)).reshape(
        KORD, N_ITILES, P, N_OTILES * 2 * P
    )

    bias_eff = bias_param + weights[:, :, 0].sum(axis=1)  # T_0 == 1 fold
    bias_t = np.ascontiguousarray(bias_eff.reshape(N_OTILES, P).T)  # [128, 8]

    # pure-pair tiles: planes (Wh_ka, Wh_kb) for itiles 0-1, Wh only
    Whf = Wh.astype(np.float32)

    def pack_pair(ka_idx, kb_idx):
        wp = np.stack([Whf[:, :, ka_idx], Whf[:, :, kb_idx]], axis=-1)
        wp = wp[:, : 2 * P, :].reshape(N_OTILES, P, 2, P, 2)  # [ot,o,itl,p,pl]
        return np.ascontiguousarray(
            wp.transpose(2, 3, 0, 4, 1).astype(f8)
        ).reshape(2, P, N_OTILES * 2 * P)

    wpair = pack_pair(3, 4)   # (k4, k5)
    wpair2 = pack_pair(5, 6)  # (k6, k7)

    in_maps = []
    for cidx in range(N_CORES):
        xc = x[cidx * B_CORE : (cidx + 1) * B_CORE]  # [512, 1024]
        xt = np.ascontiguousarray(
            xc.T.reshape(N_ITILES, P, B_CORE).transpose(1, 0, 2).reshape(
                P, N_ITILES * B_CORE
            )
        ).astype(np.float16)
        in_maps.append(
            {"xT16": xt, "wT": wT, "biasT": bias_t, "wPairT": wpair, "wPair2T": wpair2}
        )
    return in_maps


def _run(x, weights, bias_param, **spmd_kwargs):
    nc = _build()
    in_maps = _prep_inputs(x, weights, bias_param)
    res = run_bass_kernel_spmd(nc, in_maps, core_ids=list(range(N_CORES)), **spmd_kwargs)
    out = np.empty((BATCH, OUT_F), dtype=np.float32)
    for cidx in range(N_CORES):
        o = res.results[cidx]["outT"]  # [8, 128, 512] fp16
        out[cidx * B_CORE : (cidx + 1) * B_CORE] = (
            np.asarray(o).astype(np.float32).transpose(2, 0, 1).reshape(B_CORE, OUT_F)
        )
    return out, res


def kernel(x, weights, bias_param):
    out, _ = _run(x, weights, bias_param)
    return out


# revision 111
# speedup vs baseline: 1.0153x; 1.0153x over previous
"""KAN layer (Chebyshev order-7 on tanh(x)) as a Bass/Tile TRN2 kernel.

Math: out[b,o] = sum_{i,k} T_k(tanh(x[b,i])) * W[o,i,k] + bias[o],  k=0..7.

T_0 == 1 folds into an effective bias on the host. The device contracts
the remaining 7*1024 = 7168 (i,k) pairs per output.

Device strategy (data-parallel over batch, 512 rows/core):
- Basis is built on-chip in fp16: u = tanh(x), T_2 = 2u^2 - 1, then the
  even/odd Chebyshev recurrences T_{k+2} = (2 T_2) T_k - T_{k-2} as fp16
  tensor_tensor ops on the DVE (2x perf mode for 2-byte dtypes).
- The matmul runs in fp8e4 (e4m3) with DoubleRow perf mode at half a
  cycle per output row. The two DoubleRow "planes" carry a hi/lo split
  of the weights (Wh = fp8(W*2^12), Wl = fp8(W*2^12 - Wh)) against the
  same fp8 basis tile (stride-0 broadcast rhs), which cancels the
  weight-quantization error. The basis is quantized to fp8 unscaled
  (|T_k| <= ~1 sits fine in e4m3).
- psum accumulates in f32; output = psum * 2^-12 + bias_eff in fp16.
- The PE consumes chunk-major (k2..k7 of contraction-chunk a = itiles
  0-3, then chunk b) so basis production stays ahead; the last two
  levels run otile-outer with the descale+bias+store fused in,
  overlapping the drain. Dummy warm-up matmuls burn the PE p-state
  ramp before the real stream.
- Four "pure" fp8 pairs — (k4,k5) and (k6,k7) at itiles 0-1 — pack two
  k-tiles per DoubleRow instruction (planes = two basis tiles, Wh only,
  no Wl): halves those tiles' PE time and weight bytes for a measured
  rel-err cost of 0.0184 -> 0.0190 against the 0.02 gate.
"""

import sys

sys.path.insert(0, "/opt/trn_rl_repo")

import math

import ml_dtypes
import numpy as np

import concourse.bass as bass  # noqa: F401  (engine types come via bacc)
import concourse.mybir as mybir
from concourse import bacc
from concourse.bass_utils import run_bass_kernel_spmd
from concourse.tile import TileContext

P = 128
N_CORES = 8
BATCH = 4096
B_CORE = BATCH // N_CORES  # 512
IN_F = 1024
OUT_F = 1024
KORD = 7  # Chebyshev T_1..T_7 (T_0 folded into bias)
N_ITILES = IN_F // P  # 8
N_OTILES = OUT_F // P  # 8
CHUNK = 2048  # free-dim chunk: 4 itiles per chunk
N_CHUNKS = 2
SW = 4096.0  # weight fp8 scale (2^12); basis is unscaled (|T_k| <= ~1)
DESCALE = 1.0 / SW

F32 = mybir.dt.float32
F16 = mybir.dt.float16
F8 = mybir.dt.float8e4
DR = mybir.MatmulPerfMode.DoubleRow
ACT_COPY = mybir.ActivationFunctionType.Copy
ACT_TANH = mybir.ActivationFunctionType.Tanh
ACT_SQUARE = mybir.ActivationFunctionType.Square
ACT_ID = mybir.ActivationFunctionType.Identity
MULT = mybir.AluOpType.mult
SUBTRACT = mybir.AluOpType.subtract
ADD = mybir.AluOpType.add

_NC_CACHE = None


def _build():
    """Build + compile the single-core Bass program (SPMD across 8 cores)."""
    global _NC_CACHE
    if _NC_CACHE is not None:
        return _NC_CACHE

    nc = bacc.Bacc("TRN2", target_bir_lowering=False, debug=False)

    # xT16[p, it*512 + b] = x[b, it*128 + p] as fp16, for this core's slice.
    xT16 = nc.declare_dram_parameter("xT16", [P, N_ITILES * B_CORE], F16, isOutput=False)
    # wT[k', it, p, ot*256 + pl*128 + o] = {Wh,Wl}[ot*128+o, it*128+p, k'+1]
    wT = nc.declare_dram_parameter(
        "wT", [KORD, N_ITILES, P, N_OTILES * 2 * P], F8, isOutput=False
    )
    # biasT[p, ot] = bias_eff[ot*128 + p]
    biasT = nc.declare_dram_parameter("biasT", [P, N_OTILES], F32, isOutput=False)
    # Pure-fp8 pair tiles: planes = (Wh_k4, Wh_k5) for itiles 0-1 (no Wl
    # there; measured rel err 0.0184 vs the 0.02 gate). One DoubleRow
    # instruction then contracts both k-tiles at once.
    wPairT = nc.declare_dram_parameter("wPairT", [2, P, N_OTILES * 2 * P], F8, isOutput=False)
    # same for (Wh_k6, Wh_k7) at itiles 0-1
    wPair2T = nc.declare_dram_parameter("wPair2T", [2, P, N_OTILES * 2 * P], F8, isOutput=False)
    # and (Wh_k2, Wh_k3) at itile 4 (chunk b)
    wPair3T = nc.declare_dram_parameter("wPair3T", [1, P, N_OTILES * 2 * P], F8, isOutput=False)
    # outT[ot, p, b] = out[b, ot*128 + p] fp16
    outT = nc.declare_dram_parameter("outT", [N_OTILES, P, B_CORE], F16, isOutput=True)

    with TileContext(nc) as tc:
        with (
            tc.tile_pool(name="x", bufs=1) as x_pool,
            tc.tile_pool(name="chain", bufs=1) as chain_pool,
            tc.tile_pool(name="tmp", bufs=4) as tmp_pool,
            tc.tile_pool(name="b8", bufs=1) as b8_pool,
            tc.tile_pool(name="w", bufs=20) as w_pool,
            tc.tile_pool(name="osb", bufs=8) as osb_pool,
            tc.tile_pool(name="misc", bufs=1) as misc_pool,
            tc.tile_pool(name="psum", bufs=1, space="PSUM") as psum_pool,
        ):
            # x arrives in 4 pieces of [128, 1024] so tanh/cast/matmul can
            # start early; the first k=1 weight tile is interleaved between
            # x pieces on the (serial) DMA bus.
            PIECE = 1024
            N_PIECES = 4
            x_full = x_pool.tile([P, N_ITILES * B_CORE], F16, name="x_full")
            w_early = {}
            for j in range(N_PIECES):
                sl = slice(j * PIECE, (j + 1) * PIECE)
                nc.sync.dma_start(out=x_full[:, sl], in_=xT16[:, sl])
                if j <= 1:
                    wsb = w_pool.tile([P, N_OTILES * 2 * P], F8, tag="w")
                    nc.sync.dma_start(out=wsb, in_=wT[0, j, :, :])
                    w_early[j] = wsb

            bias_sb = misc_pool.tile([P, N_OTILES], F32, name="bias_sb")
            nc.sync.dma_start(out=bias_sb, in_=biasT[:, :])

            # Warm-up tile: dummy matmuls below burn the PE p-state ramp
            # (0.65/1.2 GHz for the first 3us of busy) before the real
            # stream starts, so k=1 matmuls run at the full 2.4 GHz.
            warm = misc_pool.tile([P, B_CORE], F16, name="warm")
            nc.vector.memset(warm, 0.0)

            ones = misc_pool.tile([P, CHUNK], F16, name="ones")
            nc.vector.memset(ones, 1.0)

            # ---- Chebyshev basis in fp16; fp8 casts on Act ----
            # b8[k][c] holds fp8(T_k) for itiles 4c..4c+3.
            b8 = [[None] * N_CHUNKS for _ in range(KORD + 1)]

            # (k4,k5) and (k6,k7) of chunk a each share one allocation so the
            # pure-pair matmuls can address both as DoubleRow planes
            # (plane stride 2048 within the joint tile).
            b8_45 = b8_pool.tile([P, 2 * CHUNK], F8, name="b8_45")
            b8_67 = b8_pool.tile([P, 2 * CHUNK], F8, name="b8_67")
            b8_23b = b8_pool.tile([P, 2 * CHUNK], F8, name="b8_23b")
            pair_dst = {
                (4, 0): b8_45[:, 0:CHUNK],
                (5, 0): b8_45[:, CHUNK : 2 * CHUNK],
                (6, 0): b8_67[:, 0:CHUNK],
                (7, 0): b8_67[:, CHUNK : 2 * CHUNK],
                (2, 1): b8_23b[:, 0:CHUNK],
                (3, 1): b8_23b[:, CHUNK : 2 * CHUNK],
            }

            def cast(k, c, src):
                dst = pair_dst.get((k, c))
                if dst is None:
                    dst = b8_pool.tile([P, CHUNK], F8, name=f"b8_{k}_{c}")
                nc.scalar.activation(dst, src, ACT_COPY)
                b8[k][c] = dst

            # u and b8[1] live as full-width tiles written piecewise so the
            # Act stream can emit tanh_p0, cast1_p0 before tanh_p1 etc.
            u_full = chain_pool.tile([P, N_ITILES * B_CORE], F16, name="u_full")
            b8_1 = b8_pool.tile([P, N_ITILES * B_CORE], F8, name="b8_1")
            b8[1] = [
                b8_1[:, c * CHUNK : (c + 1) * CHUNK] for c in range(N_CHUNKS)
            ]
            # Piece 0: fp8 tanh straight from x so the PE starts ASAP. fp16
            # tanh runs per piece; fp8 copies of pieces 1-3 go to the DVE.
            # Emission order is scheduling priority: sq_a sits between t16
            # pieces so only short ops can delay it, keeping the k=2 chunk-a
            # path (sq_a -> T2a halves -> cast2a halves) as early as possible.
            # Chunk b's square runs on the DVE to keep Act free for casts.
            nc.scalar.activation(b8_1[:, 0:PIECE], x_full[:, 0:PIECE], ACT_TANH)
            for j in (0, 1):
                sl = slice(j * PIECE, (j + 1) * PIECE)
                nc.scalar.activation(u_full[:, sl], x_full[:, sl], ACT_TANH)
            u = [u_full[:, c * CHUNK : (c + 1) * CHUNK] for c in range(N_CHUNKS)]

            sq_a = chain_pool.tile([P, CHUNK], F16, name="sq_a")
            nc.scalar.activation(sq_a, u[0], ACT_SQUARE, scale=math.sqrt(2.0))

            # Pieces 2-3 of the k=1 fp8 basis come straight from Act tanh;
            # their fp16 tanh is only needed by chunk-b's chain (huge slack)
            # and is emitted in the chunk-b section below.
            for j in (2, 3):
                sl = slice(j * PIECE, (j + 1) * PIECE)
                nc.scalar.activation(b8_1[:, sl], x_full[:, sl], ACT_TANH)
            nc.vector.tensor_copy(b8_1[:, PIECE : 2 * PIECE], u_full[:, PIECE : 2 * PIECE])

            # The PE consumes chunk-major (k2a..k7a then k2b..k7b), so the
            # full chunk-a chain is produced first; chunk-b has ~15us slack.
            T2, M = [None] * N_CHUNKS, [None] * N_CHUNKS
            # chunk a: T2 from the Act square, in halves for early casts
            t2a = chain_pool.tile([P, CHUNK], F16, name="T2_a")
            for h in range(2):
                hs = slice(h * PIECE, (h + 1) * PIECE)
                nc.vector.tensor_tensor(
                    out=t2a[:, hs], in0=sq_a[:, hs], in1=ones[:, hs], op=SUBTRACT
                )
            T2[0] = t2a
            b8_2 = b8_pool.tile([P, CHUNK], F8, name="b8_2_0")
            for h in range(2):
                hs = slice(h * PIECE, (h + 1) * PIECE)
                nc.scalar.activation(b8_2[:, hs], t2a[:, hs], ACT_COPY)
            b8[2][0] = b8_2

            Tk = {1: u, 2: T2}
            prev_of = {3: (1, 1), 4: (2, 0), 5: (3, 1), 6: (4, 2), 7: (5, 3)}

            # All chain DVE ops run as [1024] halves: smaller blocking
            # granularity defuses the ack-latency trap where a long op grabs
            # the engine right before a critical dependent becomes ready.
            HALVES = (slice(0, PIECE), slice(PIECE, CHUNK))

            def tt_halved(out_t, in0, in1, op):
                for hs in HALVES:
                    nc.vector.tensor_tensor(
                        out=out_t[:, hs], in0=in0[:, hs], in1=in1[:, hs], op=op
                    )

            def chain_step(k, c):
                # T_{k+2} = (2 T_2) T_k - T_{k-2}: odd from T_1, even from T_2
                src_k, sub_k = prev_of[k]
                tm = tmp_pool.tile([P, CHUNK], F16, tag="tmp")
                tt_halved(tm, M[c], Tk[src_k][c], MULT)
                tk = chain_pool.tile([P, CHUNK], F16, name=f"T{k}_{c}")
                tt_halved(tk, tm, ones if sub_k == 0 else Tk[sub_k][c], SUBTRACT)
                Tk[k].append(tk)
                cast(k, c, tk)

            for k in range(3, KORD + 1):
                Tk[k] = []

            m_a = chain_pool.tile([P, CHUNK], F16, name="M_0")
            tt_halved(m_a, t2a, t2a, ADD)
            M[0] = m_a
            for k in range(3, KORD + 1):
                chain_step(k, 0)

            # chunk b: fp16 tanh (only consumer is this chain), then
            # 2u^2 on the DVE (scalar_tensor_tensor), then -1
            for j in (2, 3):
                sl = slice(j * PIECE, (j + 1) * PIECE)
                nc.scalar.activation(u_full[:, sl], x_full[:, sl], ACT_TANH)
            d_b = chain_pool.tile([P, CHUNK], F16, name="d_b")
            for hs in HALVES:
                nc.vector.scalar_tensor_tensor(
                    out=d_b[:, hs], in0=u[1][:, hs], scalar=2.0, in1=u[1][:, hs],
                    op0=MULT, op1=MULT,
                )
            t2b = chain_pool.tile([P, CHUNK], F16, name="T2_b")
            tt_halved(t2b, d_b, ones, SUBTRACT)
            T2[1] = t2b
            cast(2, 1, t2b)
            m_b = chain_pool.tile([P, CHUNK], F16, name="M_1")
            tt_halved(m_b, t2b, t2b, ADD)
            M[1] = m_b
            for k in range(3, KORD + 1):
                chain_step(k, 1)

            # ---- DoubleRow fp8 matmuls: psum[ot] += (Wh|Wl).T @ (Bh, Bh) ----
            psums = [
                psum_pool.tile([P, B_CORE], F32, name=f"ps_{ot}")
                for ot in range(N_OTILES)
            ]
            def rhs_for(k, c, itl):
                return (
                    b8[k][c][:, itl * B_CORE : (itl + 1) * B_CORE]
                    .unsqueeze(1)
                    .broadcast_to([P, 2, B_CORE])
                )

            def lhs_for(wsb, ot):
                return wsb[:, ot * 2 * P : (ot + 1) * 2 * P].rearrange(
                    "p (two m) -> p two m", two=2
                )

            # p-state warm-up: open-and-close dummy accumulation groups on
            # bank 0; the real k=1 start=True matmul resets it afterwards.
            for _ in range(6):
                nc.tensor.matmul(
                    psums[0],
                    lhsT=warm[:, 0:P],
                    rhs=warm[:, :],
                    start=True,
                    stop=True,
                )

            ITL = N_ITILES // N_CHUNKS
            s = 0
            # k=1 for both chunks first (warm-up while the chain spins up)
            for c in range(N_CHUNKS):
                for itl in range(ITL):
                    it = c * ITL + itl
                    if it in w_early:
                        wsb = w_early[it]
                    else:
                        wsb = w_pool.tile([P, N_OTILES * 2 * P], F8, tag="w")
                        nc.sync.dma_start(out=wsb, in_=wT[0, it, :, :])
                    rhs = rhs_for(1, c, itl)
                    for ot in range(N_OTILES):
                        nc.tensor.matmul(
                            psums[ot],
                            lhsT=lhs_for(wsb, ot),
                            rhs=rhs,
                            start=(s == 0),
                            stop=False,
                            perf_mode=DR,
                        )
                    s += 1

            # chunk-major: all of chunk a's k=2..7, then chunk b's k=2..6
            def mm_block(k, c, skip=()):
                for itl in range(ITL):
                    if itl in skip:
                        continue
                    it = c * ITL + itl
                    wsb = w_pool.tile([P, N_OTILES * 2 * P], F8, tag="w")
                    nc.sync.dma_start(out=wsb, in_=wT[k - 1, it, :, :])
                    rhs = rhs_for(k, c, itl)
                    for ot in range(N_OTILES):
                        nc.tensor.matmul(
                            psums[ot],
                            lhsT=lhs_for(wsb, ot),
                            rhs=rhs,
                            start=False,
                            stop=False,
                            perf_mode=DR,
                        )

            TAIL_LEVELS = (KORD - 1, KORD)  # k=6..7 of chunk b
            w_tail = {}

            def pair_mms(joint_tile, wparam, itl):
                wp = w_pool.tile([P, N_OTILES * 2 * P], F8, tag="w")
                nc.sync.dma_start(out=wp, in_=wparam[itl, :, :])
                rhs = joint_tile[:, :].rearrange("p (two half) -> p two half", two=2)[
                    :, :, itl * B_CORE : (itl + 1) * B_CORE
                ]
                for ot in range(N_OTILES):
                    nc.tensor.matmul(
                        psums[ot],
                        lhsT=lhs_for(wp, ot),
                        rhs=rhs,
                        start=False,
                        stop=False,
                        perf_mode=DR,
                    )

            mm_block(2, 0)
            mm_block(3, 0)
            mm_block(4, 0, skip=(0, 1))
            mm_block(5, 0, skip=(0, 1))
            for itl in (0, 1):
                pair_mms(b8_45, wPairT, itl)
            mm_block(6, 0, skip=(0, 1))
            mm_block(7, 0, skip=(0, 1))
            for itl in (0, 1):
                pair_mms(b8_67, wPair2T, itl)
            mm_block(2, 1, skip=(0,))
            mm_block(3, 1, skip=(0,))
            pair_mms(b8_23b, wPair3T, 0)
            for k in range(4, TAIL_LEVELS[0]):
                mm_block(k, 1)
            for kt in TAIL_LEVELS:
                for itl in range(ITL):
                    wsb = w_pool.tile([P, N_OTILES * 2 * P], F8, tag="w")
                    nc.sync.dma_start(out=wsb, in_=wT[kt - 1, ITL + itl, :, :])
                    w_tail[(kt, itl)] = wsb

            # Final levels of chunk b run ot-outer so each psum bank finishes
            # early; the spacing lets every otile's descale+store (and its
            # serial HWDGE descriptor-gen) drain under the remaining matmuls.
            for ot in range(N_OTILES):
                for k in TAIL_LEVELS:
                    for itl in range(ITL):
                        nc.tensor.matmul(
                            psums[ot],
                            lhsT=lhs_for(w_tail[(k, itl)], ot),
                            rhs=rhs_for(k, 1, itl),
                            start=False,
                            stop=(k == KORD and itl == ITL - 1),
                            perf_mode=DR,
                        )
                # ---- descale + bias add + store (fp16) ----
                osb = osb_pool.tile([P, B_CORE], F16, tag="osb")
                nc.scalar.activation(
                    osb,
                    psums[ot],
                    ACT_ID,
                    bias=bias_sb[:, ot : ot + 1],
                    scale=DESCALE,
                )
                nc.sync.dma_start(out=outT[ot, :, :], in_=osb)

    nc.compile()
    _NC_CACHE = nc
    return _NC_CACHE


def _prep_inputs(x, weights, bias_param):
    x = np.asarray(x, dtype=np.float32)
    weights = np.asarray(weights, dtype=np.float32)
    bias_param = np.asarray(bias_param, dtype=np.float32)
    f8 = ml_dtypes.float8_e4m3

    # Weights: [o, i, k] -> hi/lo fp8 at scale 2^12, laid out
    # wT[k', it, p, ot, pl, o] with the last 3 dims contiguous (2KB lines).
    W7 = weights[:, :, 1:] * np.float32(SW)  # [o, i, 7]
    Wh = W7.astype(f8)
    Wl = (W7 - Wh.astype(np.float32)).astype(f8)
    arr = np.stack([Wh, Wl], axis=-1)  # [o_g, i_g, k, pl]
    arr = arr.reshape(N_OTILES, P, N_ITILES, P, KORD, 2)  # [ot, o, it, p, k, pl]
    wT = np.ascontiguousarray(arr.transpose(4, 2, 3, 0, 5, 1)).reshape(
        KORD, N_ITILES, P, N_OTILES * 2 * P
    )

    bias_eff = bias_param + weights[:, :, 0].sum(axis=1)  # T_0 == 1 fold
    bias_t = np.ascontiguousarray(bias_eff.reshape(N_OTILES, P).T)  # [128, 8]

    # pure-pair tiles: planes (Wh_ka, Wh_kb) for itiles 0-1, Wh only
    Whf = Wh.astype(np.float32)

    def pack_pair(ka_idx, kb_idx, it_lo=0, n_itl=2):
        wp = np.stack([Whf[:, :, ka_idx], Whf[:, :, kb_idx]], axis=-1)
        wp = wp[:, it_lo * P : (it_lo + n_itl) * P, :].reshape(
            N_OTILES, P, n_itl, P, 2
        )  # [ot,o,itl,p,pl]
        return np.ascontiguousarray(
            wp.transpose(2, 3, 0, 4, 1).astype(f8)
        ).reshape(n_itl, P, N_OTILES * 2 * P)

    wpair = pack_pair(3, 4)   # (k4, k5) itiles 0-1
    wpair2 = pack_pair(5, 6)  # (k6, k7) itiles 0-1
    wpair3 = pack_pair(1, 2, it_lo=4, n_itl=1)  # (k2, k3) itile 4

    in_maps = []
    for cidx in range(N_CORES):
        xc = x[cidx * B_CORE : (cidx + 1) * B_CORE]  # [512, 1024]
        xt = np.ascontiguousarray(
            xc.T.reshape(N_ITILES, P, B_CORE).transpose(1, 0, 2).reshape(
                P, N_ITILES * B_CORE
            )
        ).astype(np.float16)
        in_maps.append(
            {
                "xT16": xt,
                "wT": wT,
                "biasT": bias_t,
                "wPairT": wpair,
                "wPair2T": wpair2,
                "wPair3T": wpair3,
            }
        )
    return in_maps


def _run(x, weights, bias_param, **spmd_kwargs):
    nc = _build()
    in_maps = _prep_inputs(x, weights, bias_param)
    res = run_bass_kernel_spmd(nc, in_maps, core_ids=list(range(N_CORES)), **spmd_kwargs)
    out = np.empty((BATCH, OUT_F), dtype=np.float32)
    for cidx in range(N_CORES):
        o = res.results[cidx]["outT"]  # [8, 128, 512] fp16
        out[cidx * B_CORE : (cidx + 1) * B_CORE] = (
            np.asarray(o).astype(np.float32).transpose(2, 0, 1).reshape(B_CORE, OUT_F)
        )
    return out, res


def kernel(x, weights, bias_param):
    out, _ = _run(x, weights, bias_param)
    return out


# revision 114
# speedup vs baseline: 1.0179x; 1.0026x over previous
"""KAN layer (Chebyshev order-7 on tanh(x)) as a Bass/Tile TRN2 kernel.

Math: out[b,o] = sum_{i,k} T_k(tanh(x[b,i])) * W[o,i,k] + bias[o],  k=0..7.

T_0 == 1 folds into an effective bias on the host. The device contracts
the remaining 7*1024 = 7168 (i,k) pairs per output.

Device strategy (data-parallel over batch, 512 rows/core):
- Basis is built on-chip in fp16: u = tanh(x), T_2 = 2u^2 - 1, then the
  even/odd Chebyshev recurrences T_{k+2} = (2 T_2) T_k - T_{k-2} as fp16
  tensor_tensor ops on the DVE (2x perf mode for 2-byte dtypes).
- The matmul runs in fp8e4 (e4m3) with DoubleRow perf mode at half a
  cycle per output row. The two DoubleRow "planes" carry a hi/lo split
  of the weights (Wh = fp8(W*2^12), Wl = fp8(W*2^12 - Wh)) against the
  same fp8 basis tile (stride-0 broadcast rhs), which cancels the
  weight-quantization error. The basis is quantized to fp8 unscaled
  (|T_k| <= ~1 sits fine in e4m3).
- psum accumulates in f32; output = psum * 2^-12 + bias_eff in fp16.
- The PE consumes chunk-major (k2..k7 of contraction-chunk a = itiles
  0-3, then chunk b) so basis production stays ahead; the last two
  levels run otile-outer with the descale+bias+store fused in,
  overlapping the drain. Dummy warm-up matmuls burn the PE p-state
  ramp before the real stream.
- Four "pure" fp8 pairs — (k4,k5) and (k6,k7) at itiles 0-1 — pack two
  k-tiles per DoubleRow instruction (planes = two basis tiles, Wh only,
  no Wl): halves those tiles' PE time and weight bytes for a measured
  rel-err cost of 0.0184 -> 0.0190 against the 0.02 gate.
"""

import sys

sys.path.insert(0, "/opt/trn_rl_repo")

import math

import ml_dtypes
import numpy as np

import concourse.bass as bass  # noqa: F401  (engine types come via bacc)
import concourse.mybir as mybir
from concourse import bacc
from concourse.bass_utils import run_bass_kernel_spmd
from concourse.tile import TileContext

P = 128
N_CORES = 8
BATCH = 4096
B_CORE = BATCH // N_CORES  # 512
IN_F = 1024
OUT_F = 1024
KORD = 7  # Chebyshev T_1..T_7 (T_0 folded into bias)
N_ITILES = IN_F // P  # 8
N_OTILES = OUT_F // P  # 8
CHUNK = 2048  # free-dim chunk: 4 itiles per chunk
N_CHUNKS = 2
SW = 4096.0  # weight fp8 scale (2^12); basis is unscaled (|T_k| <= ~1)
DESCALE = 1.0 / SW

F32 = mybir.dt.float32
F16 = mybir.dt.float16
F8 = mybir.dt.float8e4
DR = mybir.MatmulPerfMode.DoubleRow
ACT_COPY = mybir.ActivationFunctionType.Copy
ACT_TANH = mybir.ActivationFunctionType.Tanh
ACT_SQUARE = mybir.ActivationFunctionType.Square
ACT_ID = mybir.ActivationFunctionType.Identity
MULT = mybir.AluOpType.mult
SUBTRACT = mybir.AluOpType.subtract
ADD = mybir.AluOpType.add

_NC_CACHE = None


def _build():
    """Build + compile the single-core Bass program (SPMD across 8 cores)."""
    global _NC_CACHE
    if _NC_CACHE is not None:
        return _NC_CACHE

    nc = bacc.Bacc("TRN2", target_bir_lowering=False, debug=False)

    # xT16[p, it*512 + b] = x[b, it*128 + p] as fp16, for this core's slice.
    xT16 = nc.declare_dram_parameter("xT16", [P, N_ITILES * B_CORE], F16, isOutput=False)
    # wT[k', it, p, ot*256 + pl*128 + o] = {Wh,Wl}[ot*128+o, it*128+p, k'+1]
    wT = nc.declare_dram_parameter(
        "wT", [KORD, N_ITILES, P, N_OTILES * 2 * P], F8, isOutput=False
    )
    # biasT[p, ot] = bias_eff[ot*128 + p]
    biasT = nc.declare_dram_parameter("biasT", [P, N_OTILES], F32, isOutput=False)
    # Pure-fp8 pair tiles: planes = (Wh_k4, Wh_k5) for itiles 0-1 (no Wl
    # there; measured rel err 0.0184 vs the 0.02 gate). One DoubleRow
    # instruction then contracts both k-tiles at once.
    wPairT = nc.declare_dram_parameter("wPairT", [2, P, N_OTILES * 2 * P], F8, isOutput=False)
    # same for (Wh_k6, Wh_k7) at itiles 0-1
    wPair2T = nc.declare_dram_parameter("wPair2T", [2, P, N_OTILES * 2 * P], F8, isOutput=False)
    # and (Wh_k2, Wh_k3) at itiles 4-5 (chunk b)
    wPair3T = nc.declare_dram_parameter("wPair3T", [2, P, N_OTILES * 2 * P], F8, isOutput=False)
    # outT[ot, p, b] = out[b, ot*128 + p] fp16
    outT = nc.declare_dram_parameter("outT", [N_OTILES, P, B_CORE], F16, isOutput=True)

    with TileContext(nc) as tc:
        with (
            tc.tile_pool(name="x", bufs=1) as x_pool,
            tc.tile_pool(name="chain", bufs=1) as chain_pool,
            tc.tile_pool(name="tmp", bufs=4) as tmp_pool,
            tc.tile_pool(name="b8", bufs=1) as b8_pool,
            tc.tile_pool(name="w", bufs=20) as w_pool,
            tc.tile_pool(name="osb", bufs=8) as osb_pool,
            tc.tile_pool(name="misc", bufs=1) as misc_pool,
            tc.tile_pool(name="psum", bufs=1, space="PSUM") as psum_pool,
        ):
            # x arrives in 4 pieces of [128, 1024] so tanh/cast/matmul can
            # start early; the first k=1 weight tile is interleaved between
            # x pieces on the (serial) DMA bus.
            PIECE = 1024
            N_PIECES = 4
            x_full = x_pool.tile([P, N_ITILES * B_CORE], F16, name="x_full")
            w_early = {}
            for j in range(N_PIECES):
                sl = slice(j * PIECE, (j + 1) * PIECE)
                nc.sync.dma_start(out=x_full[:, sl], in_=xT16[:, sl])
                if j <= 1:
                    wsb = w_pool.tile([P, N_OTILES * 2 * P], F8, tag="w")
                    nc.sync.dma_start(out=wsb, in_=wT[0, j, :, :])
                    w_early[j] = wsb

            bias_sb = misc_pool.tile([P, N_OTILES], F32, name="bias_sb")
            nc.sync.dma_start(out=bias_sb, in_=biasT[:, :])

            # Warm-up tile: dummy matmuls below burn the PE p-state ramp
            # (0.65/1.2 GHz for the first 3us of busy) before the real
            # stream starts, so k=1 matmuls run at the full 2.4 GHz.
            warm = misc_pool.tile([P, B_CORE], F16, name="warm")
            nc.vector.memset(warm, 0.0)

            ones = misc_pool.tile([P, CHUNK], F16, name="ones")
            nc.vector.memset(ones, 1.0)

            # ---- Chebyshev basis in fp16; fp8 casts on Act ----
            # b8[k][c] holds fp8(T_k) for itiles 4c..4c+3.
            b8 = [[None] * N_CHUNKS for _ in range(KORD + 1)]

            # (k4,k5) and (k6,k7) of chunk a each share one allocation so the
            # pure-pair matmuls can address both as DoubleRow planes
            # (plane stride 2048 within the joint tile).
            b8_45 = b8_pool.tile([P, 2 * CHUNK], F8, name="b8_45")
            b8_67 = b8_pool.tile([P, 2 * CHUNK], F8, name="b8_67")
            b8_23b = b8_pool.tile([P, 2 * CHUNK], F8, name="b8_23b")
            pair_dst = {
                (4, 0): b8_45[:, 0:CHUNK],
                (5, 0): b8_45[:, CHUNK : 2 * CHUNK],
                (6, 0): b8_67[:, 0:CHUNK],
                (7, 0): b8_67[:, CHUNK : 2 * CHUNK],
                (2, 1): b8_23b[:, 0:CHUNK],
                (3, 1): b8_23b[:, CHUNK : 2 * CHUNK],
            }

            def cast(k, c, src):
                dst = pair_dst.get((k, c))
                if dst is None:
                    dst = b8_pool.tile([P, CHUNK], F8, name=f"b8_{k}_{c}")
                nc.scalar.activation(dst, src, ACT_COPY)
                b8[k][c] = dst

            # u and b8[1] live as full-width tiles written piecewise so the
            # Act stream can emit tanh_p0, cast1_p0 before tanh_p1 etc.
            u_full = chain_pool.tile([P, N_ITILES * B_CORE], F16, name="u_full")
            b8_1 = b8_pool.tile([P, N_ITILES * B_CORE], F8, name="b8_1")
            b8[1] = [
                b8_1[:, c * CHUNK : (c + 1) * CHUNK] for c in range(N_CHUNKS)
            ]
            # Piece 0: fp8 tanh straight from x so the PE starts ASAP. fp16
            # tanh runs per piece; fp8 copies of pieces 1-3 go to the DVE.
            # Emission order is scheduling priority: sq_a sits between t16
            # pieces so only short ops can delay it, keeping the k=2 chunk-a
            # path (sq_a -> T2a halves -> cast2a halves) as early as possible.
            # Chunk b's square runs on the DVE to keep Act free for casts.
            nc.scalar.activation(b8_1[:, 0:PIECE], x_full[:, 0:PIECE], ACT_TANH)
            for j in (0, 1):
                sl = slice(j * PIECE, (j + 1) * PIECE)
                nc.scalar.activation(u_full[:, sl], x_full[:, sl], ACT_TANH)
            u = [u_full[:, c * CHUNK : (c + 1) * CHUNK] for c in range(N_CHUNKS)]

            sq_a = chain_pool.tile([P, CHUNK], F16, name="sq_a")
            nc.scalar.activation(sq_a, u[0], ACT_SQUARE, scale=math.sqrt(2.0))

            # Pieces 2-3 of the k=1 fp8 basis come straight from Act tanh;
            # their fp16 tanh is only needed by chunk-b's chain (huge slack)
            # and is emitted in the chunk-b section below.
            for j in (2, 3):
                sl = slice(j * PIECE, (j + 1) * PIECE)
                nc.scalar.activation(b8_1[:, sl], x_full[:, sl], ACT_TANH)
            nc.vector.tensor_copy(b8_1[:, PIECE : 2 * PIECE], u_full[:, PIECE : 2 * PIECE])

            # The PE consumes chunk-major (k2a..k7a then k2b..k7b), so the
            # full chunk-a chain is produced first; chunk-b has ~15us slack.
            T2, M = [None] * N_CHUNKS, [None] * N_CHUNKS
            # chunk a: T2 from the Act square, in halves for early casts
            t2a = chain_pool.tile([P, CHUNK], F16, name="T2_a")
            for h in range(2):
                hs = slice(h * PIECE, (h + 1) * PIECE)
                nc.vector.tensor_tensor(
                    out=t2a[:, hs], in0=sq_a[:, hs], in1=ones[:, hs], op=SUBTRACT
                )
            T2[0] = t2a
            b8_2 = b8_pool.tile([P, CHUNK], F8, name="b8_2_0")
            for h in range(2):
                hs = slice(h * PIECE, (h + 1) * PIECE)
                nc.scalar.activation(b8_2[:, hs], t2a[:, hs], ACT_COPY)
            b8[2][0] = b8_2

            Tk = {1: u, 2: T2}
            prev_of = {3: (1, 1), 4: (2, 0), 5: (3, 1), 6: (4, 2), 7: (5, 3)}

            # All chain DVE ops run as [1024] halves: smaller blocking
            # granularity defuses the ack-latency trap where a long op grabs
            # the engine right before a critical dependent becomes ready.
            HALVES = (slice(0, PIECE), slice(PIECE, CHUNK))

            def tt_halved(out_t, in0, in1, op):
                for hs in HALVES:
                    nc.vector.tensor_tensor(
                        out=out_t[:, hs], in0=in0[:, hs], in1=in1[:, hs], op=op
                    )

            def chain_step(k, c):
                # T_{k+2} = (2 T_2) T_k - T_{k-2}: odd from T_1, even from T_2
                src_k, sub_k = prev_of[k]
                tm = tmp_pool.tile([P, CHUNK], F16, tag="tmp")
                tt_halved(tm, M[c], Tk[src_k][c], MULT)
                tk = chain_pool.tile([P, CHUNK], F16, name=f"T{k}_{c}")
                tt_halved(tk, tm, ones if sub_k == 0 else Tk[sub_k][c], SUBTRACT)
                Tk[k].append(tk)
                cast(k, c, tk)

            for k in range(3, KORD + 1):
                Tk[k] = []

            m_a = chain_pool.tile([P, CHUNK], F16, name="M_0")
            tt_halved(m_a, t2a, t2a, ADD)
            M[0] = m_a
            for k in range(3, KORD + 1):
                chain_step(k, 0)

            # chunk b: fp16 tanh (only consumer is this chain), then
            # 2u^2 on the DVE (scalar_tensor_tensor), then -1
            for j in (2, 3):
                sl = slice(j * PIECE, (j + 1) * PIECE)
                nc.scalar.activation(u_full[:, sl], x_full[:, sl], ACT_TANH)
            d_b = chain_pool.tile([P, CHUNK], F16, name="d_b")
            for hs in HALVES:
                nc.vector.scalar_tensor_tensor(
                    out=d_b[:, hs], in0=u[1][:, hs], scalar=2.0, in1=u[1][:, hs],
                    op0=MULT, op1=MULT,
                )
            t2b = chain_pool.tile([P, CHUNK], F16, name="T2_b")
            tt_halved(t2b, d_b, ones, SUBTRACT)
            T2[1] = t2b
            cast(2, 1, t2b)
            m_b = chain_pool.tile([P, CHUNK], F16, name="M_1")
            tt_halved(m_b, t2b, t2b, ADD)
            M[1] = m_b
            for k in range(3, KORD + 1):
                chain_step(k, 1)

            # ---- DoubleRow fp8 matmuls: psum[ot] += (Wh|Wl).T @ (Bh, Bh) ----
            psums = [
                psum_pool.tile([P, B_CORE], F32, name=f"ps_{ot}")
                for ot in range(N_OTILES)
            ]
            def rhs_for(k, c, itl):
                return (
                    b8[k][c][:, itl * B_CORE : (itl + 1) * B_CORE]
                    .unsqueeze(1)
                    .broadcast_to([P, 2, B_CORE])
                )

            def lhs_for(wsb, ot):
                return wsb[:, ot * 2 * P : (ot + 1) * 2 * P].rearrange(
                    "p (two m) -> p two m", two=2
                )

            # p-state warm-up: open-and-close dummy accumulation groups on
            # bank 0; the real k=1 start=True matmul resets it afterwards.
            for _ in range(6):
                nc.tensor.matmul(
                    psums[0],
                    lhsT=warm[:, 0:P],
                    rhs=warm[:, :],
                    start=True,
                    stop=True,
                )

            ITL = N_ITILES // N_CHUNKS
            s = 0
            # k=1 for both chunks first (warm-up while the chain spins up)
            for c in range(N_CHUNKS):
                for itl in range(ITL):
                    it = c * ITL + itl
                    if it in w_early:
                        wsb = w_early[it]
                    else:
                        wsb = w_pool.tile([P, N_OTILES * 2 * P], F8, tag="w")
                        nc.sync.dma_start(out=wsb, in_=wT[0, it, :, :])
                    rhs = rhs_for(1, c, itl)
                    for ot in range(N_OTILES):
                        nc.tensor.matmul(
                            psums[ot],
                            lhsT=lhs_for(wsb, ot),
                            rhs=rhs,
                            start=(s == 0),
                            stop=False,
                            perf_mode=DR,
                        )
                    s += 1

            # chunk-major: all of chunk a's k=2..7, then chunk b's k=2..6
            def mm_block(k, c, skip=()):
                for itl in range(ITL):
                    if itl in skip:
                        continue
                    it = c * ITL + itl
                    wsb = w_pool.tile([P, N_OTILES * 2 * P], F8, tag="w")
                    nc.sync.dma_start(out=wsb, in_=wT[k - 1, it, :, :])
                    rhs = rhs_for(k, c, itl)
                    for ot in range(N_OTILES):
                        nc.tensor.matmul(
                            psums[ot],
                            lhsT=lhs_for(wsb, ot),
                            rhs=rhs,
                            start=False,
                            stop=False,
                            perf_mode=DR,
                        )

            TAIL_LEVELS = (KORD - 1, KORD)  # k=6..7 of chunk b
            w_tail = {}

            def pair_mms(joint_tile, wparam, itl):
                wp = w_pool.tile([P, N_OTILES * 2 * P], F8, tag="w")
                nc.sync.dma_start(out=wp, in_=wparam[itl, :, :])
                rhs = joint_tile[:, :].rearrange("p (two half) -> p two half", two=2)[
                    :, :, itl * B_CORE : (itl + 1) * B_CORE
                ]
                for ot in range(N_OTILES):
                    nc.tensor.matmul(
                        psums[ot],
                        lhsT=lhs_for(wp, ot),
                        rhs=rhs,
                        start=False,
                        stop=False,
                        perf_mode=DR,
                    )

            mm_block(2, 0)
            mm_block(3, 0)
            mm_block(4, 0, skip=(0, 1))
            mm_block(5, 0, skip=(0, 1))
            for itl in (0, 1):
                pair_mms(b8_45, wPairT, itl)
            mm_block(6, 0, skip=(0, 1))
            mm_block(7, 0, skip=(0, 1))
            for itl in (0, 1):
                pair_mms(b8_67, wPair2T, itl)
            mm_block(2, 1, skip=(0, 1))
            mm_block(3, 1, skip=(0, 1))
            pair_mms(b8_23b, wPair3T, 0)
            pair_mms(b8_23b, wPair3T, 1)
            for k in range(4, TAIL_LEVELS[0]):
                mm_block(k, 1)
            for kt in TAIL_LEVELS:
                for itl in range(ITL):
                    wsb = w_pool.tile([P, N_OTILES * 2 * P], F8, tag="w")
                    nc.sync.dma_start(out=wsb, in_=wT[kt - 1, ITL + itl, :, :])
                    w_tail[(kt, itl)] = wsb

            # Final levels of chunk b run ot-outer so each psum bank finishes
            # early; the spacing lets every otile's descale+store (and its
            # serial HWDGE descriptor-gen) drain under the remaining matmuls.
            for ot in range(N_OTILES):
                for k in TAIL_LEVELS:
                    for itl in range(ITL):
                        nc.tensor.matmul(
                            psums[ot],
                            lhsT=lhs_for(w_tail[(k, itl)], ot),
                            rhs=rhs_for(k, 1, itl),
                            start=False,
                            stop=(k == KORD and itl == ITL - 1),
                            perf_mode=DR,
                        )
                # ---- descale + bias add + store (fp16) ----
                osb = osb_pool.tile([P, B_CORE], F16, tag="osb")
                nc.scalar.activation(
                    osb,
                    psums[ot],
                    ACT_ID,
                    bias=bias_sb[:, ot : ot + 1],
                    scale=DESCALE,
                )
                nc.sync.dma_start(out=outT[ot, :, :], in_=osb)

    nc.compile()
    _NC_CACHE = nc
    return _NC_CACHE


def _prep_inputs(x, weights, bias_param):
    x = np.asarray(x, dtype=np.float32)
    weights = np.asarray(weights, dtype=np.float32)
    bias_param = np.asarray(bias_param, dtype=np.float32)
    f8 = ml_dtypes.float8_e4m3

    # Weights: [o, i, k] -> hi/lo fp8 at scale 2^12, laid out
    # wT[k', it, p, ot, pl, o] with the last 3 dims contiguous (2KB lines).
    W7 = weights[:, :, 1:] * np.float32(SW)  # [o, i, 7]
    Wh = W7.astype(f8)
    Wl = (W7 - Wh.astype(np.float32)).astype(f8)
    arr = np.stack([Wh, Wl], axis=-1)  # [o_g, i_g, k, pl]
    arr = arr.reshape(N_OTILES, P, N_ITILES, P, KORD, 2)  # [ot, o, it, p, k, pl]
    wT = np.ascontiguousarray(arr.transpose(4, 2, 3, 0, 5, 1)).reshape(
        KORD, N_ITILES, P, N_OTILES * 2 * P
    )

    bias_eff = bias_param + weights[:, :, 0].sum(axis=1)  # T_0 == 1 fold
    bias_t = np.ascontiguousarray(bias_eff.reshape(N_OTILES, P).T)  # [128, 8]

    # pure-pair tiles: planes (Wh_ka, Wh_kb) for itiles 0-1, Wh only
    Whf = Wh.astype(np.float32)

    def pack_pair(ka_idx, kb_idx, it_lo=0, n_itl=2):
        wp = np.stack([Whf[:, :, ka_idx], Whf[:, :, kb_idx]], axis=-1)
        wp = wp[:, it_lo * P : (it_lo + n_itl) * P, :].reshape(
            N_OTILES, P, n_itl, P, 2
        )  # [ot,o,itl,p,pl]
        return np.ascontiguousarray(
            wp.transpose(2, 3, 0, 4, 1).astype(f8)
        ).reshape(n_itl, P, N_OTILES * 2 * P)

    wpair = pack_pair(3, 4)   # (k4, k5) itiles 0-1
    wpair2 = pack_pair(5, 6)  # (k6, k7) itiles 0-1
    wpair3 = pack_pair(1, 2, it_lo=4, n_itl=2)  # (k2, k3) itiles 4-5

    in_maps = []
    for cidx in range(N_CORES):
        xc = x[cidx * B_CORE : (cidx + 1) * B_CORE]  # [512, 1024]
        xt = np.ascontiguousarray(
            xc.T.reshape(N_ITILES, P, B_CORE).transpose(1, 0, 2).reshape(
                P, N_ITILES * B_CORE
            )
        ).astype(np.float16)
        in_maps.append(
            {
                "xT16": xt,
                "wT": wT,
                "biasT": bias_t,
                "wPairT": wpair,
                "wPair2T": wpair2,
                "wPair3T": wpair3,
            }
        )
    return in_maps


def _run(x, weights, bias_param, **spmd_kwargs):
    nc = _build()
    in_maps = _prep_inputs(x, weights, bias_param)
    res = run_bass_kernel_spmd(nc, in_maps, core_ids=list(range(N_CORES)), **spmd_kwargs)
    out = np.empty((BATCH, OUT_F), dtype=np.float32)
    for cidx in range(N_CORES):
        o = res.results[cidx]["outT"]  # [8, 128, 512] fp16
        out[cidx * B_CORE : (cidx + 1) * B_CORE] = (
            np.asarray(o).astype(np.float32).transpose(2, 0, 1).reshape(B_CORE, OUT_F)
        )
    return out, res


def kernel(x, weights, bias_param):
    out, _ = _run(x, weights, bias_param)
    return out
